# revision 22
# baseline (speedup 1.0000x reference)
"""GPT forward pass on 8 Trainium2 NeuronCores.

Sharding: DP=2 over batch x TP=4 within each group of 4 cores.
 - attention: 3 heads per core; MLP: 768 of 3072 hidden per core
 - per layer: 2 AllReduces (attn-out, mlp-out), each split into two
   512-token halves (bf16 payload) so the collective overlaps compute
 - tied output projection sharded over (padded) vocab: 12800 cols/core

All matmul operands are bf16 (PSUM accumulation stays fp32); the fp32
residual stream lives in SBUF in (T-part, E-free) layout. LN stats via
bn_stats on DVE; the normalize (x-mu)*rstd runs on ScalarE as
Identity(scale*x+bias) with per-partition scale/bias. h is transposed
to (E-part, T-free) bf16 slabs via PE transposes. V is produced
directly in (token-part, head-dim) layout by using Wv as the moving
operand. Softmax skips max-subtraction (scores ~N(0,0.3), 1/sqrt(D)
folded into Wq); denominators come from a ones-column appended to V;
the reciprocal runs on ScalarE over a PE-broadcast (64,512) tile.
Logits are written as bf16 and upcast on the host.
"""

import sys

sys.path.insert(0, "/opt/trn_rl_repo")

import math

import numpy as np
import ml_dtypes

import concourse.bass as bass
import concourse.mybir as mybir
import concourse.tile as tile
from concourse import bacc
from concourse.bass_utils import run_bass_kernel_spmd

V, E, H, L, T = 50257, 768, 12, 6, 1024
B = 2
D = E // H  # 64
EPS = 1e-5

NCORES = 8
TP = 4
HPC = H // TP               # 3 heads per core
ECOLS = HPC * D             # 192 qkv out cols per core
QK = 2 * ECOLS              # 384 q+k cols per core
HID = 4 * E // TP           # 768 mlp hidden per core
VSH = 12800                 # padded vocab shard per core
VPAD = VSH * TP

KO = E // 128               # 6
NT = T // 128               # 8
NQ = T // 512               # 2
HC = HID // 128             # 6
NV = VSH // 512             # 25

F32 = mybir.dt.float32
BF16 = mybir.dt.bfloat16
BF = ml_dtypes.bfloat16

REPLICA_GROUPS = [[0, 1, 2, 3], [4, 5, 6, 7]]

_CACHE = {}


def _build(flags, dbg=None):
    has_qkv_b, has_b1c, has_bo, has_b2, has_lb = flags
    nc = bacc.Bacc("TRN2", target_bir_lowering=False, debug=False,
                   num_devices=NCORES)

    io = {}
    io["x0"] = nc.declare_dram_parameter("x0", [T, E], BF16, isOutput=False)
    io["qkw"] = nc.declare_dram_parameter("qkw", [L, 128, KO, QK], BF16,
                                          isOutput=False)
    io["wv"] = nc.declare_dram_parameter("wv", [L, 128, KO, ECOLS], BF16,
                                         isOutput=False)
    io["wo"] = nc.declare_dram_parameter("wo", [L, 128, 2, E], BF16,
                                         isOutput=False)
    io["w1"] = nc.declare_dram_parameter("w1", [L, 128, KO, HID], BF16,
                                         isOutput=False)
    io["w2"] = nc.declare_dram_parameter("w2", [L, 128, HC, E], BF16,
                                         isOutput=False)
    io["wteT"] = nc.declare_dram_parameter("wteT", [NV, 128, KO, 512], BF16,
                                           isOutput=False)
    io["qkvb"] = nc.declare_dram_parameter("qkvb", [L, 3, 128], F32,
                                           isOutput=False)
    io["vbias"] = nc.declare_dram_parameter("vbias", [L, 128, ECOLS], F32,
                                            isOutput=False)
    io["bo"] = nc.declare_dram_parameter("bo", [L, E], F32, isOutput=False)
    io["b1c"] = nc.declare_dram_parameter("b1c", [L, HC, 128], F32,
                                          isOutput=False)
    io["b2"] = nc.declare_dram_parameter("b2", [L, E], F32, isOutput=False)
    io["lb"] = nc.declare_dram_parameter("lb", [VSH], F32, isOutput=False)
    io["mask"] = nc.declare_dram_parameter("mask", [128, 4, 512], BF16,
                                           isOutput=False)
    io["ident"] = nc.declare_dram_parameter("ident", [128, 128], BF16,
                                            isOutput=False)
    io["ones64"] = nc.declare_dram_parameter("ones64", [65, 64], BF16,
                                             isOutput=False)
    io["logits"] = nc.declare_dram_parameter("logits", [T, VSH], BF16,
                                             isOutput=True)

    with tile.TileContext(nc) as tc:
        with tc.tile_pool(name="state", bufs=1) as state, \
             tc.tile_pool(name="act", bufs=2) as act, \
             tc.tile_pool(name="qkvp", bufs=1) as qkvp, \
             tc.tile_pool(name="wts", bufs=2) as wts, \
             tc.tile_pool(name="wvp", bufs=2) as wvp, \
             tc.tile_pool(name="small", bufs=3) as small, \
             tc.tile_pool(name="ppT", bufs=3) as ppT, \
             tc.tile_pool(name="ps", bufs=3, space="PSUM") as ppool, \
             tc.tile_pool(name="ps_acc", bufs=3, space="PSUM") as pacc, \
             tc.tile_pool(name="ps_big", bufs=2, space="PSUM") as pbig, \
             tc.tile_pool(name="dram", bufs=2, space="DRAM") as dram:

            pools = dict(state=state, act=act, qkvp=qkvp, wts=wts, wvp=wvp,
                         small=small, ppT=ppT, ps=ppool, pacc=pacc,
                         pbig=pbig, dram=dram)
            _emit(nc, tc, io, pools, flags, dbg)
    nc.finalize()
    return nc


def _emit(nc, tc, io, pools, flags, dbg=None):
    has_qkv_b, has_b1c, has_bo, has_b2, has_lb = flags
    state, act, qkvp = pools["state"], pools["act"], pools["qkvp"]
    wts, wvp = pools["wts"], pools["wvp"]
    small, ppT = pools["small"], pools["ppT"]
    ppool, pacc, pbig = pools["ps"], pools["pacc"], pools["pbig"]
    dram = pools["dram"]
    AF = mybir.ActivationFunctionType

    # ---- persistent state ----
    x_sb = state.tile([128, NT, E], BF16)
    ident = state.tile([128, 128], BF16)
    ones64 = state.tile([65, 64], BF16)
    mask_sb = state.tile([128, 4, 512], BF16)
    eps_sb = state.tile([128, 1], F32)
    oT_sb = state.tile([128, 2, T], BF16)
    V_sb = state.tile([128, NT, HPC, 65], BF16)

    nc.sync.dma_start(ident[:], io["ident"][:])
    nc.sync.dma_start(ones64[:], io["ones64"][:])
    nc.sync.dma_start(mask_sb[:], io["mask"][:])
    nc.vector.memset(eps_sb[:], EPS)
    nc.vector.memset(oT_sb[64:128, 1, :], 0.0)
    nc.vector.memset(V_sb[:, :, :, 64:65], 1.0)
    warm_sb = state.tile([128, 512], BF16)
    nc.vector.memset(warm_sb[:], 0.5)
    x0r = io["x0"].rearrange("(t p) e -> p t e", p=128)
    nc.sync.dma_start(x_sb[:, 0:4, :], x0r[:, 0:4, :])
    nc.sync.dma_start(x_sb[:, 4:8, :], x0r[:, 4:8, :])

    def warm(src_ap):
        pw = pbig.tile([128, 512], F32, tag="ps_big", name="pwarm")
        nc.tensor.matmul(pw[:], ident[:], src_ap, start=True, stop=True)

    def dump_x():
        for t in range(NT):
            nc.sync.dma_start(io["logits"][t * 128:(t + 1) * 128, 0:E],
                              x_sb[:, t, :])

    if dbg == "x0":
        dump_x()
        return

    def ln_tile(dst_T, t):
        """LN of x_sb tile t -> transposed bf16 slab columns of dst_T."""
        src = x_sb[:, t, :]
        stats = small.tile([128, 3, 6], F32, tag="ln_stats")
        for sg in range(3):
            nc.vector.bn_stats(stats[:, sg, :],
                               src[:, sg * 256:(sg + 1) * 256])
        mv = small.tile([128, 2], F32, tag="ln_mv")
        nc.vector.bn_aggr(mv[:], stats[:])
        rstd = small.tile([128, 1], F32, tag="ln_rstd")
        nc.scalar.activation(rstd[:], mv[:, 1:2], AF.Sqrt,
                             bias=eps_sb[:], scale=1.0)
        nc.vector.reciprocal(rstd[:], rstd[:])
        negb = small.tile([128, 1], F32, tag="ln_negb")
        nc.vector.tensor_scalar(negb[:], mv[:, 0:1], rstd[:], -1.0,
                                op0=mybir.AluOpType.mult,
                                op1=mybir.AluOpType.mult)
        h = small.tile([128, E], BF16, tag="ln_h")
        nc.scalar.activation(h[:], src, AF.Identity,
                             bias=negb[:], scale=rstd[:])
        for ko in range(KO):
            pt = ppool.tile([128, 1024], BF16, tag="ps_a")
            nc.tensor.transpose(pt[:, 0:128],
                                h[:, ko * 128:(ko + 1) * 128], ident[:])
            nc.vector.tensor_copy(dst_T[:, ko, t * 128:(t + 1) * 128],
                                  pt[:, 0:128])

    def res_ln(cc_out, half, dst_T, nm):
        """AR output is the new x (residual folded into the AR): load + LN."""
        nc.sync.dma_start(x_sb[:, 4 * half:4 * half + 4, :],
                          cc_out.rearrange("(t p) e -> p t e", p=128))
        for tt in range(4):
            ln_tile(dst_T, 4 * half + tt)

    # chunk -> (dst tile idx, slot, dst rows, ps rows); 0=qT 1=kT
    chunk_dsts = [
        [(0, 0, slice(0, 128), slice(0, 128))],
        [(0, 1, slice(0, 64), slice(0, 64)),
         (1, 0, slice(0, 64), slice(64, 128))],
        [(1, 0, slice(64, 128), slice(0, 64)),
         (1, 1, slice(0, 64), slice(64, 128))],
    ]

    def qkv_half(qt, hT, qT, kT, qkw, wv, qkvb, vb):
        """q,k projections for token half qt + V tiles for this half."""
        qs = slice(qt * 512, (qt + 1) * 512)
        for ci in range(3):
            ps = ppool.tile([128, 512], F32, tag="ps_a")
            for ko in range(KO):
                nc.tensor.matmul(ps[:], qkw[:, ko, ci * 128:ci * 128 + 128],
                                 hT[:, ko, qs], start=(ko == 0),
                                 stop=(ko == KO - 1))
            for (dsti, slot, drows, prows) in chunk_dsts[ci]:
                dstt = qT if dsti == 0 else kT
                if has_qkv_b:
                    nc.scalar.activation(dstt[drows, slot, qs],
                                         ps[prows, :], AF.Identity,
                                         bias=qkvb[prows, ci:ci + 1],
                                         scale=1.0)
                else:
                    nc.vector.tensor_copy(dstt[drows, slot, qs],
                                          ps[prows, :])
        for t in range(4 * qt, 4 * qt + 4):
            ts_ = slice(t * 128, (t + 1) * 128)
            pm = ppool.tile([128, 512], F32, tag="ps_a")
            for ko in range(KO):
                nc.tensor.matmul(pm[:, :ECOLS], hT[:, ko, ts_],
                                 wv[:, ko, :], start=(ko == 0),
                                 stop=(ko == KO - 1))
            if has_qkv_b:
                nc.vector.tensor_add(pm[:, :ECOLS], pm[:, :ECOLS], vb[:])
            for h in range(HPC):
                nc.vector.tensor_copy(V_sb[:, t, h, 0:64],
                                      pm[:, h * 64:(h + 1) * 64])

    def attn_half(l, qt, qT, kT, wo, bo_bc, cc_in, cc_out):
        """attention for query half qt (causal wedges), out-proj, AR."""
        qs = slice(qt * 512, (qt + 1) * 512)
        nk = 4 * qt + 4
        pos = []
        for h in range(HPC):
            slot, hp = h // 2, h % 2
            rows = slice(64 * hp, 64 * hp + 64)
            po = pacc.tile([128, 512], F32, tag="ps_acc",
                           name=f"po{h}_{l}_{qt}")
            pos.append(po)
            for kc in range(nk):
                ks = slice(kc * 128, (kc + 1) * 128)
                masked = kc >= 4 * qt
                w0 = (kc - 4 * qt) * 128 if masked else 0
                ws = slice(w0, 512)
                qws = slice(qt * 512 + w0, (qt + 1) * 512)
                pss = ppool.tile([128, 512], F32, tag="ps_a")
                nc.tensor.matmul(pss[:, ws], kT[rows, slot, ks],
                                 qT[rows, slot, qws], start=True,
                                 stop=not masked)
                if masked:
                    nc.tensor.matmul(pss[:, ws], ident[:],
                                     mask_sb[:, kc - 4 * qt, ws],
                                     start=False, stop=True)
                pT = ppT.tile([128, 512], BF16, tag="pT")
                nc.scalar.activation(pT[:, ws], pss[:, ws], AF.Exp)
                nc.tensor.matmul(po[0:65, ws], V_sb[:, kc, h, :],
                                 pT[:, ws], start=(kc == 0),
                                 stop=(kc == nk - 1))
        dall = small.tile([65, 512], F32, tag="dall")
        for h in range(HPC):
            nc.vector.tensor_copy(dall[32 * h:32 * h + 1, :],
                                  pos[h][64:65, :])
        rall = small.tile([65, 512], BF16, tag="rall")
        with nc.allow_low_precision(reason="softmax denom recip"):
            nc.vector.reciprocal(rall[0:65, :], dall[0:65, :])
        for h in range(HPC):
            slot, hp = h // 2, h % 2
            rows = slice(64 * hp, 64 * hp + 64)
            pb = ppool.tile([128, 512], F32, tag="ps_a")
            nc.tensor.matmul(pb[0:64, :], ones64[32 * h:32 * h + 1, :],
                             rall[32 * h:32 * h + 1, :],
                             start=True, stop=True)
            rb = small.tile([64, 512], BF16, tag="rb")
            nc.scalar.activation(rb[:], pb[0:64, :], AF.Identity)
            nc.vector.tensor_mul(oT_sb[rows, slot, qs], pos[h][0:64, :],
                                 rb[:])
        for tt in range(4):
            t = 4 * qt + tt
            ts_ = slice(t * 128, (t + 1) * 128)
            for n0, nw in ((0, 512), (512, 256)):
                pp = pbig.tile([128, 512], F32, tag="ps_big")
                for slot in range(2):
                    nc.tensor.matmul(pp[:, :nw], oT_sb[:, slot, ts_],
                                     wo[:, slot, n0:n0 + nw],
                                     start=(slot == 0), stop=(slot == 1))
                ob = small.tile([128, 512], BF16, tag="ebuf")
                nc.vector.scalar_tensor_tensor(
                    ob[:, :nw], x_sb[:, t, n0:n0 + nw], 0.25, pp[:, :nw],
                    op0=mybir.AluOpType.mult, op1=mybir.AluOpType.add)
                if has_bo:
                    nc.vector.tensor_add(ob[:, :nw], ob[:, :nw],
                                         bo_bc[:, n0:n0 + nw])
                nc.sync.dma_start(cc_in[tt * 128:(tt + 1) * 128,
                                        n0:n0 + nw], ob[:, :nw])
        nc.gpsimd.collective_compute(
            "AllReduce", mybir.AluOpType.add, replica_groups=REPLICA_GROUPS,
            ins=[cc_in.opt()], outs=[cc_out.opt()])

    def mlp_half(qt, h2T, w1, w2, b1c, b2_bc, cc_in, cc_out):
        qs = slice(qt * 512, (qt + 1) * 512)
        m1T = act.tile([128, HC, 512], BF16, tag="m1T")
        for hc in range(HC):
            pm = pacc.tile([128, 512], F32, tag="ps_acc")
            for ko in range(KO):
                nc.tensor.matmul(pm[:], w1[:, ko, hc * 128:(hc + 1) * 128],
                                 h2T[:, ko, qs],
                                 start=(ko == 0), stop=(ko == KO - 1))
            if has_b1c:
                nc.scalar.activation(m1T[:, hc, :], pm[:], AF.Gelu,
                                     bias=b1c[:, hc:hc + 1], scale=1.0)
            else:
                nc.scalar.activation(m1T[:, hc, :], pm[:], AF.Gelu)
        for tt in range(4):
            ts_ = slice(tt * 128, (tt + 1) * 128)
            for n0, nw in ((0, 512), (512, 256)):
                pp = pbig.tile([128, 512], F32, tag="ps_big")
                for hc in range(HC):
                    nc.tensor.matmul(
                        pp[:, :nw], m1T[:, hc, ts_],
                        w2[:, hc, n0:n0 + nw], start=(hc == 0),
                        stop=(hc == HC - 1))
                mb = small.tile([128, 512], BF16, tag="ebuf")
                nc.vector.scalar_tensor_tensor(
                    mb[:, :nw], x_sb[:, 4 * qt + tt, n0:n0 + nw], 0.25,
                    pp[:, :nw],
                    op0=mybir.AluOpType.mult, op1=mybir.AluOpType.add)
                if has_b2:
                    nc.vector.tensor_add(mb[:, :nw], mb[:, :nw],
                                         b2_bc[:, n0:n0 + nw])
                nc.sync.dma_start(cc_in[tt * 128:(tt + 1) * 128,
                                        n0:n0 + nw], mb[:, :nw])
        nc.gpsimd.collective_compute(
            "AllReduce", mybir.AluOpType.add, replica_groups=REPLICA_GROUPS,
            ins=[cc_in.opt()], outs=[cc_out.opt()])

    pend_m = None  # previous layer's MLP AR outputs, applied lazily per half
    for l in range(L):
        hT = act.tile([128, KO, T], BF16, tag="hT", name=f"hT_{l}")
        qT = qkvp.tile([128, 2, T], BF16, tag="qT", name=f"qT_{l}")
        kT = qkvp.tile([128, 2, T], BF16, tag="kT", name=f"kT_{l}")
        qkw = wts.tile([128, KO, QK], BF16, tag="qkw", name=f"qkw_{l}")
        nc.sync.dma_start(qkw[:], io["qkw"][l])
        wv = wts.tile([128, KO, ECOLS], BF16, tag="wv", name=f"wv_{l}")
        nc.sync.dma_start(wv[:], io["wv"][l])
        wo = wts.tile([128, 2, E], BF16, tag="wo", name=f"wo_{l}")
        nc.sync.dma_start(wo[:], io["wo"][l])
        qkvb = vb = bo_bc = None
        if has_qkv_b:
            qkvb = wts.tile([128, 3], F32, tag="qkvb")
            nc.sync.dma_start(qkvb[:], io["qkvb"][l].rearrange("c p -> p c"))
            vb = wts.tile([128, ECOLS], F32, tag="vb")
            nc.sync.dma_start(vb[:], io["vbias"][l])
        if has_bo:
            bo_bc = wts.tile([128, E], F32, tag="bo_bc")
            nc.gpsimd.dma_start(bo_bc[:], bass.AP(
                tensor=io["bo"], offset=io["bo"][l].offset,
                ap=[[0, 128], [1, E]]))
        cc_a_in = [dram.tile([512, E], BF16, tag=f"cc_a_in{hf}",
                             name=f"cc_a_in{hf}_{l}") for hf in range(2)]
        cc_a_out = [dram.tile([512, E], BF16, tag=f"cc_a_out{hf}",
                              name=f"cc_a_out{hf}_{l}") for hf in range(2)]

        for half in range(2):
            if pend_m is not None:
                res_ln(pend_m[half], half, hT, f"m{l}_{half}")
            else:
                for t in range(4 * half, 4 * half + 4):
                    ln_tile(hT, t)
            qkv_half(half, hT, qT, kT, qkw, wv, qkvb, vb)
            attn_half(l, half, qT, kT, wo, bo_bc,
                      cc_a_in[half], cc_a_out[half])

        w1 = wts.tile([128, KO, HID], BF16, tag="w1", name=f"w1_{l}")
        nc.sync.dma_start(w1[:], io["w1"][l])
        w2 = wts.tile([128, HC, E], BF16, tag="w2", name=f"w2_{l}")
        nc.sync.dma_start(w2[:], io["w2"][l])
        b1c = b2_bc = None
        if has_b1c:
            b1c = wts.tile([128, HC], F32, tag="b1c")
            nc.sync.dma_start(b1c[:], io["b1c"][l].rearrange("c p -> p c"))
        if has_b2:
            b2_bc = wts.tile([128, E], F32, tag="b2_bc")
            nc.gpsimd.dma_start(b2_bc[:], bass.AP(
                tensor=io["b2"], offset=io["b2"][l].offset,
                ap=[[0, 128], [1, E]]))
        cc_m_in = [dram.tile([512, E], BF16, tag=f"cc_m_in{hf}",
                             name=f"cc_m_in{hf}_{l}") for hf in range(2)]
        cc_m_out = [dram.tile([512, E], BF16, tag=f"cc_m_out{hf}",
                              name=f"cc_m_out{hf}_{l}") for hf in range(2)]
        h2T = act.tile([128, KO, T], BF16, tag="hT", name=f"h2T_{l}")
        for half in range(2):
            res_ln(cc_a_out[half], half, h2T, f"a{l}_{half}")
            mlp_half(half, h2T, w1, w2, b1c, b2_bc,
                     cc_m_in[half], cc_m_out[half])
        pend_m = cc_m_out

    # ---- final residual + LN + tied projection over vocab shard ----
    xfT = act.tile([128, KO, T], BF16, tag="hT")
    for half in range(2):
        res_ln(pend_m[half], half, xfT, f"f_{half}")
    if dbg == "xf":
        dump_x()
        return

    for half in range(2):
        for vt in range(NV):
            vs = slice(vt * 512, (vt + 1) * 512)
            wvt = wvp.tile([128, KO, 512], BF16, tag=f"wvt{half}",
                           name=f"wvt{half}_{vt}")
            nc.sync.dma_start(wvt[:], io["wteT"][vt])
            lb_bc = None
            if has_lb:
                lb_bc = small.tile([128, 512], F32, tag="lb_bc")
                nc.gpsimd.dma_start(lb_bc[:], bass.AP(
                    tensor=io["lb"], offset=io["lb"][vs].offset,
                    ap=[[0, 128], [1, 512]]))
            for t in range(4 * half, 4 * half + 4):
                ts_ = slice(t * 128, (t + 1) * 128)
                pl = pbig.tile([128, 512], F32, tag="ps_big")
                for ko in range(KO):
                    nc.tensor.matmul(pl[:], xfT[:, ko, ts_], wvt[:, ko, :],
                                     start=(ko == 0), stop=(ko == KO - 1))
                lg = small.tile([128, 512], BF16, tag="lg")
                if has_lb:
                    nc.vector.tensor_add(lg[:], pl[:], lb_bc[:])
                else:
                    nc.vector.tensor_copy(lg[:], pl[:])
                nc.sync.dma_start(io["logits"][ts_, vs], lg[:])


def _prep_inputs(inputs):
    tokens = np.asarray(inputs["tokens"])
    wte = np.asarray(inputs["wte"], dtype=np.float32)
    wpe = np.asarray(inputs["wpe"], dtype=np.float32)
    Wq = np.asarray(inputs["Wq"], dtype=np.float32)
    Wk = np.asarray(inputs["Wk"], dtype=np.float32)
    Wv = np.asarray(inputs["Wv"], dtype=np.float32)
    Wo = np.asarray(inputs["Wo"], dtype=np.float32)
    bo = np.asarray(inputs["bo"], dtype=np.float32)
    ln1_s = np.asarray(inputs["ln1_s"], dtype=np.float32)
    ln1_b = np.asarray(inputs["ln1_b"], dtype=np.float32)
    W1 = np.asarray(inputs["W1"], dtype=np.float32)
    b1 = np.asarray(inputs["b1"], dtype=np.float32)
    W2 = np.asarray(inputs["W2"], dtype=np.float32)
    b2 = np.asarray(inputs["b2"], dtype=np.float32)
    ln2_s = np.asarray(inputs["ln2_s"], dtype=np.float32)
    ln2_b = np.asarray(inputs["ln2_b"], dtype=np.float32)
    lnf_s = np.asarray(inputs["lnf_s"], dtype=np.float32)
    lnf_b = np.asarray(inputs["lnf_b"], dtype=np.float32)

    x0 = wte[tokens] + wpe[:T][None, :, :]

    scale = 1.0 / math.sqrt(D)
    Wq_f = ln1_s[:, :, None] * Wq * scale
    Wk_f = ln1_s[:, :, None] * Wk
    Wv_f = ln1_s[:, :, None] * Wv
    W1_f = ln2_s[:, :, None] * W1

    mask = np.zeros((128, 4, 512), dtype=np.float32)
    for j in range(4):
        for r in range(128):
            kpos = j * 128 + r
            if kpos > 0:
                mask[r, j, :kpos] = -1e9
    mask = mask.astype(BF)
    ident = np.eye(128, dtype=BF)
    ones64 = np.ones((65, 64), dtype=BF)

    wteT = np.ascontiguousarray(lnf_s[:, None] * wte.T)
    wteT_pad = np.zeros((E, VPAD), dtype=np.float32)
    wteT_pad[:, :V] = wteT
    lb_full = (lnf_b @ wteT_pad if np.any(lnf_b)
               else np.zeros(VPAD, np.float32)).astype(np.float32)

    flags = (bool(np.any(ln1_b)), bool(np.any(ln2_b) or np.any(b1)),
             bool(np.any(bo)), bool(np.any(b2)), bool(np.any(lnf_b)))

    per_core = []
    for c in range(NCORES):
        g, r = divmod(c, TP)
        ecl = slice(r * ECOLS, (r + 1) * ECOLS)
        hsl = slice(r * HID, (r + 1) * HID)
        vsl = slice(r * VSH, (r + 1) * VSH)

        # q,k weights: (L, E, 384) -> (L, 128, KO, 384)
        qk = np.concatenate([Wq_f[:, :, ecl], Wk_f[:, :, ecl]], axis=2)
        qk = qk.reshape(L, KO, 128, QK).transpose(0, 2, 1, 3)

        qkvb = np.zeros((L, 3, 128), dtype=np.float32)
        vbias = np.zeros((L, 128, ECOLS), dtype=np.float32)
        if flags[0]:
            qb = np.stack([ln1_b[l] @ np.concatenate(
                [Wq_f[l][:, ecl], Wk_f[l][:, ecl]], axis=1)
                for l in range(L)])  # (L, 384)
            qkvb = np.ascontiguousarray(qb.reshape(L, 3, 128))
            vbias = np.broadcast_to(
                np.stack([ln1_b[l] @ Wv_f[l][:, ecl] for l in range(L)]
                         )[:, None, :], (L, 128, ECOLS)).copy()

        wv_sh = Wv_f[:, :, ecl].reshape(L, KO, 128, ECOLS).transpose(0, 2, 1, 3)

        wo_sh = np.zeros((L, 128, 2, E), dtype=np.float32)
        wo_rows = Wo[:, ecl, :]
        wo_sh[:, :, 0, :] = wo_rows[:, 0:128]
        wo_sh[:, :64, 1, :] = wo_rows[:, 128:192]

        w1_sh = W1_f[:, :, hsl].reshape(L, KO, 128, HID).transpose(0, 2, 1, 3)
        b1c = np.zeros((L, HC, 128), dtype=np.float32)
        if flags[1]:
            bb = np.stack([ln2_b[l] @ W1_f[l][:, hsl] + b1[l, hsl]
                           for l in range(L)])
            b1c = np.ascontiguousarray(bb.reshape(L, HC, 128))
        w2_sh = W2[:, hsl, :].reshape(L, HC, 128, E).transpose(0, 2, 1, 3)

        wteT_sh = wteT_pad[:, vsl].reshape(KO, 128, NV, 512).transpose(
            2, 1, 0, 3)

        per_core.append({
            "x0": np.ascontiguousarray(x0[g].astype(BF)),
            "qkw": np.ascontiguousarray(qk.astype(BF)),
            "wv": np.ascontiguousarray(wv_sh.astype(BF)),
            "wo": np.ascontiguousarray(wo_sh.astype(BF)),
            "w1": np.ascontiguousarray(w1_sh.astype(BF)),
            "w2": np.ascontiguousarray(w2_sh.astype(BF)),
            "wteT": np.ascontiguousarray(wteT_sh.astype(BF)),
            "qkvb": qkvb,
            "vbias": vbias,
            "bo": (bo / TP).astype(np.float32),
            "b1c": b1c,
            "b2": (b2 / TP).astype(np.float32),
            "lb": np.ascontiguousarray(lb_full[vsl]),
            "mask": mask,
            "ident": ident,
            "ones64": ones64,
        })
    return per_core, flags


def kernel(**inputs) -> np.ndarray:
    per_core, flags = _prep_inputs(inputs)
    if flags not in _CACHE:
        _CACHE[flags] = _build(flags)
    nc = _CACHE[flags]
    res = run_bass_kernel_spmd(nc, per_core, list(range(NCORES)))
    kernel._nc = nc
    kernel._in_maps = per_core
    kernel._res = res

    out = np.empty((B, T, V), dtype=np.float32)
    for g in range(B):
        full = np.concatenate(
            [res.results[g * TP + r]["logits"] for r in range(TP)], axis=1)
        out[g] = full[:, :V].astype(np.float32)
    return out
